# revision 14
# baseline (speedup 1.0000x reference)
"""Trainium2 Bass kernel for nn_BertClassifier (span classifier + frame-id head).

Contract: kernel(**inputs) takes the FULL unsharded inputs (as produced by the
reference setup) and returns the full outputs (results [64009, 2, 40],
results_fid [2, 700]), matching reference.reference(**inputs).

Sharding: span-start rows i (253, padded to 256) are split 32-per-core across
8 NeuronCores. Each core computes, for its rows and both batch elements,
  h[d, (i,j)] = relu(AjT[d, j] + AiT[d, i] + ind(i,j)*w_ind[d] + b1[d])
  logitsT = W2T_k-tiles @ h  (PSUM accumulate over 7 k-tiles of D1=770)
followed by an on-chip transpose, log-softmax along the 40-way label axis and
the span_avail masking. The tiny frame-id matmul is sharded over its 700
output columns (88 per core); its log-softmax runs on host.
"""

import numpy as np

import concourse.bass as bass
import concourse.mybir as mybir
from concourse import bacc
from concourse.tile import TileContext
from concourse.bass_utils import run_bass_kernel_spmd

F32 = mybir.dt.float32
F32R = mybir.dt.float32r
I32 = mybir.dt.int32
AL = mybir.AluOpType
AF = mybir.ActivationFunctionType

M = 253          # real span rows/cols
B = 2
NH = 768         # BERT hidden
D1 = 770         # span MLP hidden
OUT = 40
FID = 700
RPC = 32         # rows per core
NCORE = 8
HK = 6           # 768 / 128 k-tiles for the A-matmuls
NDT = 7          # d-tiles of D1 (6x128 + 2)
DT_W = [128, 128, 128, 128, 128, 128, 2]
KAUG = 2 * NH + FID + 1   # 2237 rows of augmented fid input (with ones row)
FKT = 18         # ceil(2237/128)
FOUT = 88        # fid output columns per core (8*88 = 704 >= 700)
SAFE = 253       # dyn-slice start that lands in pad space

_CACHE: dict = {}


def _build(wb: int, parts=("p1", "fid", "p2", "dyn", "post")):
    """Build + compile the (uniform, SPMD) Bacc program for correction-width
    bucket `wb`."""
    hblk = 256 + wb  # free-dim width of one row block inside an H tile

    nc = bacc.Bacc()

    tokt = nc.declare_dram_parameter("TOKT", [B, HK, 128, 256], F32R, isOutput=False)
    toki = nc.declare_dram_parameter("TOKI", [B, HK, 128, RPC], F32R, isOutput=False)
    w1a = nc.declare_dram_parameter("W1A", [HK, 128, D1], F32R, isOutput=False)
    w1b = nc.declare_dram_parameter("W1B", [HK, 128, D1], F32R, isOutput=False)
    w2t = nc.declare_dram_parameter("W2T", [128, NDT, OUT], F32R, isOutput=False)
    wind = nc.declare_dram_parameter("WIND", [128, NDT], F32, isOutput=False)
    b1c = nc.declare_dram_parameter("B1C", [128, NDT], F32, isOutput=False)
    avp = nc.declare_dram_parameter("AVP", [16, 128, 4], F32, isOutput=False)
    inpt = nc.declare_dram_parameter("INPT", [128, FKT, 2], F32R, isOutput=False)
    wfids = nc.declare_dram_parameter("WFIDS", [128, FKT, FOUT], F32R, isOutput=False)
    idn = nc.declare_dram_parameter("IDN", [128, OUT], F32R, isOutput=False)
    meta = nc.declare_dram_parameter("META", [1, 128], I32, isOutput=False)

    outr = nc.declare_dram_parameter("OUTR", [RPC * 254, B, OUT], F32, isOutput=True)
    outf = nc.declare_dram_parameter("OUTF", [2, FOUT], F32, isOutput=True)

    with TileContext(nc) as tc:
        with tc.tile_pool(name="const", bufs=1) as cp, \
             tc.tile_pool(name="work", bufs=2) as wp, \
             tc.tile_pool(name="post", bufs=2) as pp:

            # ---- constant loads -------------------------------------------
            tokt_sb = [[cp.tile([128, 256], F32R, name=f"tokt{b}{k}", tag=f"tokt{b}{k}")
                        for k in range(HK)] for b in range(B)]
            toki_sb = [[cp.tile([128, RPC], F32R, name=f"toki{b}{k}", tag=f"toki{b}{k}")
                        for k in range(HK)] for b in range(B)]
            w1a_sb = [cp.tile([128, D1], F32R, name=f"w1a{k}", tag=f"w1a{k}") for k in range(HK)]
            w1b_sb = [cp.tile([128, D1], F32R, name=f"w1b{k}", tag=f"w1b{k}") for k in range(HK)]
            for b in range(B):
                for k in range(HK):
                    nc.sync.dma_start(out=tokt_sb[b][k][:, :], in_=tokt[b, k])
                    nc.sync.dma_start(out=toki_sb[b][k][:, :], in_=toki[b, k])
            for k in range(HK):
                nc.sync.dma_start(out=w1a_sb[k][:, :], in_=w1a[k])
                nc.sync.dma_start(out=w1b_sb[k][:, :], in_=w1b[k])
            w2t_sb = cp.tile([128, NDT, OUT], F32R, name="w2t", tag="w2t")
            nc.sync.dma_start(out=w2t_sb[:, :, :], in_=w2t[:, :, :])
            wind_sb = cp.tile([128, NDT], F32, name="wind", tag="wind")
            nc.sync.dma_start(out=wind_sb[:, :], in_=wind[:, :])
            b1c_sb = cp.tile([128, NDT], F32, name="b1c", tag="b1c")
            nc.sync.dma_start(out=b1c_sb[:, :], in_=b1c[:, :])
            inpt_sb = cp.tile([128, FKT, 2], F32R, name="inpt", tag="inpt")
            nc.sync.dma_start(out=inpt_sb[:, :, :], in_=inpt[:, :, :])
            wfids_sb = cp.tile([128, FKT, FOUT], F32R, name="wfids", tag="wfids")
            nc.sync.dma_start(out=wfids_sb[:, :, :], in_=wfids[:, :, :])
            idn_sb = cp.tile([128, OUT], F32R, name="idn", tag="idn")
            nc.sync.dma_start(out=idn_sb[:, :], in_=idn[:, :])
            meta_sb = cp.tile([1, 128], I32, name="meta", tag="meta")
            nc.sync.dma_start(out=meta_sb[:, :], in_=meta[:, :])

            # ---- phase 1: AjT / AjC / A0 per (batch, d-tile) --------------
            ajt = [[None] * NDT for _ in range(B)]
            ajc = [[None] * NDT for _ in range(B)]
            a0 = [[None] * NDT for _ in range(B)]

            e1v = [None, None]
            if "dyn" in parts or "dyn1" in parts:
                e1regs = [nc.vector.alloc_register(f"e1r{b}") for b in range(B)]
                for b in range(B):
                    nc.vector.reg_load(e1regs[b], meta_sb[0:1, 64 + b:65 + b])
                    e1v[b] = nc.vector.snap(e1regs[b], min_val=0, max_val=253)

            with tc.tile_pool(name="ps1", bufs=2, space="PSUM") as ps1:
                for b in range(B):
                    for dt in range(NDT):
                        dw = DT_W[dt]
                        dwe = dw + 1 if dt == NDT - 1 else dw  # incl. ones row
                        d0 = 128 * dt
                        pa = ps1.tile([128, 256], F32, name="pa", tag="pa")
                        for k in range(HK):
                            nc.tensor.matmul(
                                pa[0:dw, 0:256],
                                w1b_sb[k][:, d0:d0 + dw],
                                tokt_sb[b][k][:, 0:256],
                                start=(k == 0), stop=(k == HK - 1))
                        t_ajt = cp.tile([128, 512], F32, name=f"ajt{b}{dt}", tag=f"ajt{b}{dt}")
                        if dt == NDT - 1:
                            nc.vector.memset(t_ajt[0:dwe, 0:512], 0.0)
                        else:
                            nc.vector.memset(t_ajt[0:dw, 256:512], 0.0)
                        nc.scalar.activation(t_ajt[0:dw, 0:256], pa[0:dw, 0:256],
                                             AF.Copy)
                        ajt[b][dt] = t_ajt

                        t_ajc = cp.tile([128, 512], F32, name=f"ajc{b}{dt}", tag=f"ajc{b}{dt}")
                        nc.vector.tensor_scalar(
                            t_ajc[0:dwe, 0:253], t_ajt[0:dwe, 0:253],
                            wind_sb[0:dwe, dt:dt + 1], None, AL.add)
                        nc.vector.memset(t_ajc[0:dwe, 253:512], 0.0)
                        if "dyn" in parts or "dyn1" in parts:
                            nc.vector.tensor_copy(
                                t_ajc[0:dwe, bass.ds(e1v[b], 254)],
                                t_ajt[0:dwe, bass.ds(e1v[b], 254)])
                        ajc[b][dt] = t_ajc

                        pi = ps1.tile([128, RPC], F32, name="pi", tag="pi")
                        for k in range(HK):
                            nc.tensor.matmul(
                                pi[0:dw, 0:RPC],
                                w1a_sb[k][:, d0:d0 + dw],
                                toki_sb[b][k][:, 0:RPC],
                                start=(k == 0), stop=(k == HK - 1))
                        t_a0 = cp.tile([128, RPC], F32, name=f"a0{b}{dt}", tag=f"a0{b}{dt}")
                        if dt == NDT - 1:
                            nc.vector.memset(t_a0[0:dwe, :], 1.0)
                        nc.vector.tensor_scalar(
                            t_a0[0:dw, :], pi[0:dw, :],
                            b1c_sb[0:dw, dt:dt + 1], None, AL.add)
                        a0[b][dt] = t_a0

            # ---- phase 3 emitted early so fid matmuls fill PE bubbles -----
            _ps2cm = tc.tile_pool(name="ps2", bufs=2, space="PSUM")
            ps2 = _ps2cm.__enter__()
            if "fid" not in parts:
                psf = None
            psf = ps2.tile([2, FOUT], F32, name="psf", tag="psf", bufs=1) if "fid" in parts else None
            if "fid" in parts:
                for k in range(FKT):
                    nc.tensor.matmul(psf[:, :], inpt_sb[:, k, :], wfids_sb[:, k, :],
                                     start=(k == 0), stop=(k == FKT - 1))
                outf_sb = pp.tile([2, FOUT], F32, name="outf", tag="outf")
                nc.vector.tensor_copy(outf_sb[:, :], psf[:, :])
                nc.sync.dma_start(out=outf[:, :], in_=outf_sb[:, :])

            # ---- phase 2: span units (2 rows x 253 spans, per batch) ------
            ACT_DTS = (0, 1)  # op1a d-tiles handled by the scalar engine
            # two DVE registers reused by every unit's dynamic-slice loads
            vregs = [nc.vector.alloc_register(f"vdyn{rr}") for rr in range(2)]
            units = range(16) if "p2" in parts else range(0)
            for u in units:
                for b in range(B):
                    hts = [wp.tile([128, 2, hblk], F32R, name=f"h{dt}", tag=f"h{dt}")
                           for dt in range(NDT)]
                    vload = [None, None]
                    if "dyn" in parts or "dyn2" in parts:
                        for rr in range(2):
                            r = 2 * u + rr
                            nc.vector.reg_load(
                                vregs[rr],
                                meta_sb[0:1, b * RPC + r:b * RPC + r + 1])
                            vload[rr] = nc.vector.snap(
                                vregs[rr], min_val=0, max_val=253)
                    for rr in range(2):
                        r = 2 * u + rr
                        for dt in range(NDT):
                            dwe = DT_W[dt] + 1 if dt == NDT - 1 else DT_W[dt]
                            dst = hts[dt][0:dwe, rr, 0:254]
                            src = ajt[b][dt][0:dwe, 0:254]
                            bias = a0[b][dt][0:dwe, r:r + 1]
                            if dt in ACT_DTS:
                                nc.scalar.activation(dst, src, AF.Relu, bias=bias)
                            else:
                                nc.vector.tensor_scalar(dst, src, bias, 0.0,
                                                        AL.add, AL.max)
                        if "dyn" in parts or "dyn2" in parts:
                            for dt in range(NDT):
                                dwe = DT_W[dt] + 1 if dt == NDT - 1 else DT_W[dt]
                                nc.vector.tensor_scalar(
                                    hts[dt][0:dwe, rr, bass.ds(vload[rr], wb)],
                                    ajc[b][dt][0:dwe, bass.ds(vload[rr], wb)],
                                    a0[b][dt][0:dwe, r:r + 1], 0.0,
                                    AL.add, AL.max)

                    psl = ps2.tile([OUT, 508], F32, name="psl", tag="psl")
                    for dt in range(NDT):
                        kw = DT_W[dt] + 1 if dt == NDT - 1 else DT_W[dt]
                        nc.tensor.matmul(
                            psl[:, :],
                            w2t_sb[0:kw, dt, :],
                            hts[dt][0:kw, :, 0:254],
                            start=(dt == 0), stop=(dt == NDT - 1))

                    sl = pp.tile([OUT, 512], F32R, name="sl", tag="sl")
                    nc.scalar.activation(sl[0:OUT, 0:508], psl[:, :], AF.Copy)

                    pst = ps2.tile([128, 4 * OUT], F32R, name="pst", tag="pst")
                    for ck in range(4):
                        cw = 128 if ck < 3 else 508 - 3 * 128
                        nc.tensor.transpose(
                            pst[0:cw, OUT * ck:OUT * (ck + 1)],
                            sl[0:OUT, 128 * ck:128 * ck + cw],
                            idn_sb[0:OUT, 0:OUT])

                    if "post" not in parts:
                        continue
                    se = pp.tile([128, 4], F32, name="se", tag="se")
                    nc.vector.memset(se[:, :], 1.0)
                    et = pp.tile([128, 4 * OUT], F32, name="et", tag="et")
                    for ck in range(4):
                        cw = 128 if ck < 3 else 508 - 3 * 128
                        nc.scalar.activation(
                            et[0:cw, OUT * ck:OUT * (ck + 1)],
                            pst[0:cw, OUT * ck:OUT * (ck + 1)],
                            AF.Exp, accum_out=se[0:cw, ck:ck + 1])
                    lns = pp.tile([128, 4], F32, name="lns", tag="lns")
                    nc.scalar.activation(lns[:, :], se[:, :], AF.Ln)

                    av = pp.tile([128, 4], F32, name="av", tag="av")
                    nc.sync.dma_start(out=av[:, :], in_=avp[u])
                    t1 = pp.tile([128, 4], F32, name="t1", tag="t1")
                    nc.vector.tensor_tensor(t1[:, :], av[:, :], lns[:, :], AL.mult)
                    s1 = pp.tile([128, 4], F32, name="s1", tag="s1")
                    nc.vector.tensor_tensor(s1[:, :], av[:, :], t1[:, :],
                                            AL.subtract)
                    nc.vector.tensor_scalar(s1[:, :], s1[:, :], -1.0, None, AL.add)

                    ot = pp.tile([128, 4 * OUT], F32, name="ot", tag="ot")
                    for ck in range(4):
                        cw = 128 if ck < 3 else 508 - 3 * 128
                        nc.vector.tensor_scalar(
                            ot[0:cw, OUT * ck:OUT * (ck + 1)],
                            pst[0:cw, OUT * ck:OUT * (ck + 1)],
                            av[0:cw, ck:ck + 1], s1[0:cw, ck:ck + 1],
                            AL.mult, AL.add)
                    base = 508 * u
                    for ck in range(4):
                        cw = 128 if ck < 3 else 508 - 3 * 128
                        nc.sync.dma_start(
                            out=outr[base + 128 * ck:base + 128 * ck + cw, b, :],
                            in_=ot[0:cw, OUT * ck:OUT * (ck + 1)])

            _ps2cm.__exit__(None, None, None)

    nc.compile()
    return nc


def _host_prep(hidden, pred_spans, token_nums, span_avail, fid_vecs,
               W1, b1, W2, b2, Wfid, bfid):
    f32 = np.float32
    hidden = np.ascontiguousarray(hidden, dtype=f32)
    span_avail = np.asarray(span_avail)
    fid_vecs = np.asarray(fid_vecs, dtype=f32)
    W1 = np.asarray(W1, dtype=f32)
    b1 = np.asarray(b1, dtype=f32)
    W2 = np.asarray(W2, dtype=f32)
    b2 = np.asarray(b2, dtype=f32)
    Wfid = np.asarray(Wfid, dtype=f32)
    bfid = np.asarray(bfid, dtype=f32)
    s0, e0 = int(pred_spans[0, 0]), int(pred_spans[0, 1])
    s1_, e1_ = int(pred_spans[1, 0]), int(pred_spans[1, 1])
    spans = [(s0, e0), (s1_, e1_)]

    # correction width bucket
    wneed = 1
    for (s, e) in spans:
        jhi = min(e, M - 1)
        if 0 <= s <= jhi:
            wneed = max(wneed, jhi - s + 1)
    wb = 8
    while wb < wneed:
        wb *= 2
    wb = min(wb, 256)

    tok = hidden[:, 1:M + 1, :]                       # [2, 253, 768]
    tokt_full = np.zeros((B, NH, 256), f32)
    tokt_full[:, :, :M] = np.transpose(tok, (0, 2, 1))
    TOKT = np.ascontiguousarray(
        tokt_full.reshape(B, HK, 128, 256))

    W1A = np.ascontiguousarray(W1[0:NH].reshape(HK, 128, D1))
    W1B = np.ascontiguousarray(W1[NH:2 * NH].reshape(HK, 128, D1))
    w_ind = W1[2 * NH]                                # [770]

    W2T = np.zeros((128, NDT, OUT), f32)
    for dt in range(6):
        W2T[:, dt, :] = W2[128 * dt:128 * (dt + 1), :]
    W2T[0, 6, :] = W2[768, :]
    W2T[1, 6, :] = W2[769, :]
    W2T[2, 6, :] = b2

    WIND = np.zeros((128, NDT), f32)
    B1C = np.zeros((128, NDT), f32)
    for dt in range(NDT):
        dw = DT_W[dt]
        WIND[:dw, dt] = w_ind[128 * dt:128 * dt + dw]
        B1C[:dw, dt] = b1[128 * dt:128 * dt + dw]

    IDN = np.zeros((128, OUT), f32)
    IDN[np.arange(OUT), np.arange(OUT)] = 1.0

    # fid head input (host): cls, masked mean, fid_vecs, ones
    Lpad = hidden.shape[1]
    p = np.arange(Lpad)
    t = np.asarray(token_nums).astype(np.int64)
    dis = np.array([e0 - s0, e1_ - s1_], np.int64)
    mask = ((p[None, :] >= t[:, None] + 2)
            & (p[None, :] <= t[:, None] + 2 + dis[:, None])).astype(np.float64)
    denom = mask.sum(axis=1, keepdims=True)
    pred_vecs = (np.einsum('bl,blh->bh', mask, hidden.astype(np.float64))
                 / denom)
    cls = hidden[:, 0, :].astype(np.float64)
    inp_aug = np.concatenate(
        [cls, pred_vecs, fid_vecs.astype(np.float64),
         np.ones((B, 1))], axis=1).astype(f32)       # [2, 2237]
    INPT = np.zeros((128, FKT, 2), f32)
    for k in range(FKT):
        rows = inp_aug[:, 128 * k:128 * (k + 1)]      # [2, <=128]
        INPT[:rows.shape[1], k, :] = rows.T

    Wfid_aug = np.concatenate([Wfid, bfid[None, :]], axis=0)  # [2237, 700]

    # per-core tensors
    av_flat = (span_avail.reshape(-1) == 1)

    per_core = []
    for c in range(NCORE):
        gi0 = RPC * c
        TOKI = np.zeros((B, HK, 128, RPC), f32)
        for r in range(RPC):
            gi = gi0 + r
            if gi < M:
                TOKI[:, :, :, r] = hidden[:, gi + 1, :].reshape(B, HK, 128)

        META = np.zeros((1, 128), np.int32)
        META[0, :] = SAFE
        for b, (s, e) in enumerate(spans):
            jhi = min(e, M - 1)
            for r in range(RPC):
                gi = gi0 + r
                v = gi if (s <= gi <= jhi) else SAFE
                META[0, b * RPC + r] = v
            META[0, 64 + b] = max(0, min(e + 1, M))

        AVP = np.zeros((16, 128, 4), f32)
        for u in range(16):
            for ck in range(4):
                cw = 128 if ck < 3 else 508 - 384
                for pi in range(cw):
                    ls = 128 * ck + pi          # position within the 508-col unit
                    rr, j = ls // 254, ls % 254
                    gi = gi0 + 2 * u + rr
                    if j < M and gi < M and av_flat[gi * M + j]:
                        AVP[u, pi, ck] = 1.0

        WFIDS = np.zeros((128, FKT, FOUT), f32)
        cols = Wfid_aug[:, FOUT * c:min(FOUT * (c + 1), FID)]  # [2237, <=88]
        for k in range(FKT):
            rows = cols[128 * k:128 * (k + 1)]
            WFIDS[:rows.shape[0], k, :rows.shape[1]] = rows

        per_core.append(dict(
            TOKT=TOKT, TOKI=TOKI, W1A=W1A, W1B=W1B, W2T=W2T, WIND=WIND,
            B1C=B1C, AVP=AVP, INPT=INPT, WFIDS=WFIDS, IDN=IDN, META=META))

    aux = dict(spans=spans, tok=tok, w_ind=w_ind, W1=W1, b1=b1, W2=W2, b2=b2,
               av_flat=av_flat)
    return wb, per_core, aux


def _exact_cell_fix(results, aux):
    """The device computes the exact span cell (s_b, e_b) with ind=1 (inside).
    Recompute it exactly on host with ind=2 and overwrite."""
    tok = aux["tok"].astype(np.float64)
    W1 = aux["W1"].astype(np.float64)
    for b, (s, e) in enumerate(aux["spans"]):
        if not (0 <= s < M and 0 <= e < M):
            continue
        n = s * M + e
        if not aux["av_flat"][n]:
            continue
        ai = tok[b, s] @ W1[0:NH]
        aj = tok[b, e] @ W1[NH:2 * NH]
        h = ai + aj + 2.0 * aux["w_ind"].astype(np.float64) \
            + aux["b1"].astype(np.float64)
        h = np.maximum(h, 0.0)
        logits = h @ aux["W2"].astype(np.float64) + aux["b2"].astype(np.float64)
        mx = logits.max()
        logp = logits - (mx + np.log(np.exp(logits - mx).sum()))
        results[n, b, :] = logp.astype(np.float32)


def kernel(hidden, pred_spans, token_nums, span_avail, fid_vecs,
           W1, b1, W2, b2, Wfid, bfid):
    wb, per_core, aux = _host_prep(
        hidden, pred_spans, token_nums, span_avail, fid_vecs,
        W1, b1, W2, b2, Wfid, bfid)

    if wb not in _CACHE:
        _CACHE[wb] = _build(wb)
    nc = _CACHE[wb]

    res = run_bass_kernel_spmd(nc, per_core, list(range(NCORE)))

    parts = []
    for c in range(NCORE):
        o = res.results[c]["OUTR"].reshape(RPC, 254, B, OUT)[:, :M]
        parts.append(o.reshape(RPC * M, B, OUT))
    results = np.ascontiguousarray(
        np.concatenate(parts, axis=0)[:M * M])
    _exact_cell_fix(results, aux)

    fl = np.concatenate(
        [res.results[c]["OUTF"] for c in range(NCORE)], axis=1)[:, :FID]
    fl64 = fl.astype(np.float64)
    mx = fl64.max(axis=1, keepdims=True)
    lse = mx + np.log(np.exp(fl64 - mx).sum(axis=1, keepdims=True))
    results_fid = (fl64 - lse).astype(np.float32)

    return results, results_fid


# revision 15
# speedup vs baseline: 1.1171x; 1.1171x over previous
"""Trainium2 Bass kernel for nn_BertClassifier (span classifier + frame-id head).

Contract: kernel(**inputs) takes the FULL unsharded inputs (as produced by the
reference setup) and returns the full outputs (results [64009, 2, 40],
results_fid [2, 700]), matching reference.reference(**inputs).

Sharding: span-start rows i (253, padded to 256) are split 32-per-core across
8 NeuronCores. Each core computes, for its rows and both batch elements,
  h[d, (i,j)] = relu(AjT[d, j] + AiT[d, i] + ind(i,j)*w_ind[d] + b1[d])
  logitsT = W2T_k-tiles @ h  (PSUM accumulate over 7 k-tiles of D1=770)
followed by an on-chip transpose, log-softmax along the 40-way label axis and
the span_avail masking. The tiny frame-id matmul is sharded over its 700
output columns (88 per core); its log-softmax runs on host.

The h pipeline runs in bf16 (PE streams 1 col/cycle, DVE hits its 4x mode);
accumulation stays fp32 in PSUM. The span indicator corrections use
host-prepared dynamic-slice offsets (META) consumed via reused DVE registers.
"""

import numpy as np
import ml_dtypes

import concourse.bass as bass
import concourse.mybir as mybir
from concourse import bacc
from concourse.tile import TileContext
from concourse.bass_utils import run_bass_kernel_spmd

F32 = mybir.dt.float32
F32R = mybir.dt.float32r
BF16 = mybir.dt.bfloat16
I32 = mybir.dt.int32
AL = mybir.AluOpType
AF = mybir.ActivationFunctionType

M = 253          # real span rows/cols
B = 2
NH = 768         # BERT hidden
D1 = 770         # span MLP hidden
OUT = 40
FID = 700
RPC = 32         # rows per core
NCORE = 8
HK = 6           # 768 / 128 k-tiles for the A-matmuls
NDT = 7          # d-tiles of D1 (6x128 + 2)
DT_W = [128, 128, 128, 128, 128, 128, 2]
FKT = 18         # ceil(2237/128)
FOUT = 88        # fid output columns per core (8*88 = 704 >= 700)
SAFE = 253       # dyn-slice start that lands in pad space

_CACHE: dict = {}


def _build(wb: int, parts=("p1", "fid", "p2", "dyn", "post")):
    """Build + compile the (uniform, SPMD) Bacc program for correction-width
    bucket `wb`."""
    hblk = 256 + wb  # free-dim width of one row block inside an H tile

    nc = bacc.Bacc()

    tokt = nc.declare_dram_parameter("TOKT", [B, HK, 128, 256], BF16, isOutput=False)
    toki = nc.declare_dram_parameter("TOKI", [B, HK, 128, RPC], BF16, isOutput=False)
    w1a = nc.declare_dram_parameter("W1A", [HK, 128, D1], BF16, isOutput=False)
    w1b = nc.declare_dram_parameter("W1B", [HK, 128, D1], BF16, isOutput=False)
    w2t = nc.declare_dram_parameter("W2T", [128, NDT, OUT], BF16, isOutput=False)
    wind = nc.declare_dram_parameter("WIND", [128, NDT], F32, isOutput=False)
    b1c = nc.declare_dram_parameter("B1C", [128, NDT], F32, isOutput=False)
    avp = nc.declare_dram_parameter("AVP", [16, 128, 4], F32, isOutput=False)
    inpt = nc.declare_dram_parameter("INPT", [128, FKT, 2], BF16, isOutput=False)
    wfids = nc.declare_dram_parameter("WFIDS", [128, FKT, FOUT], BF16, isOutput=False)
    idn = nc.declare_dram_parameter("IDN", [128, OUT], F32R, isOutput=False)
    meta = nc.declare_dram_parameter("META", [1, 128], I32, isOutput=False)

    outr = nc.declare_dram_parameter("OUTR", [RPC * 254, B, OUT], F32, isOutput=True)
    outf = nc.declare_dram_parameter("OUTF", [2, FOUT], F32, isOutput=True)

    with TileContext(nc) as tc:
        with tc.tile_pool(name="const", bufs=1) as cp, \
             tc.tile_pool(name="work", bufs=2) as wp, \
             tc.tile_pool(name="post", bufs=2) as pp:

            # ---- constant loads -------------------------------------------
            tokt_sb = [[cp.tile([128, 256], BF16, name=f"tokt{b}{k}", tag=f"tokt{b}{k}")
                        for k in range(HK)] for b in range(B)]
            toki_sb = [[cp.tile([128, RPC], BF16, name=f"toki{b}{k}", tag=f"toki{b}{k}")
                        for k in range(HK)] for b in range(B)]
            w1a_sb = [cp.tile([128, D1], BF16, name=f"w1a{k}", tag=f"w1a{k}")
                      for k in range(HK)]
            w1b_sb = [cp.tile([128, D1], BF16, name=f"w1b{k}", tag=f"w1b{k}")
                      for k in range(HK)]
            for b in range(B):
                for k in range(HK):
                    nc.sync.dma_start(out=tokt_sb[b][k][:, :], in_=tokt[b, k])
                    nc.sync.dma_start(out=toki_sb[b][k][:, :], in_=toki[b, k])
            for k in range(HK):
                nc.sync.dma_start(out=w1a_sb[k][:, :], in_=w1a[k])
                nc.sync.dma_start(out=w1b_sb[k][:, :], in_=w1b[k])
            w2t_sb = cp.tile([128, NDT, OUT], BF16, name="w2t", tag="w2t")
            nc.sync.dma_start(out=w2t_sb[:, :, :], in_=w2t[:, :, :])
            wind_sb = cp.tile([128, NDT], F32, name="wind", tag="wind")
            nc.sync.dma_start(out=wind_sb[:, :], in_=wind[:, :])
            b1c_sb = cp.tile([128, NDT], F32, name="b1c", tag="b1c")
            nc.sync.dma_start(out=b1c_sb[:, :], in_=b1c[:, :])
            inpt_sb = cp.tile([128, FKT, 2], BF16, name="inpt", tag="inpt")
            nc.sync.dma_start(out=inpt_sb[:, :, :], in_=inpt[:, :, :])
            wfids_sb = cp.tile([128, FKT, FOUT], BF16, name="wfids", tag="wfids")
            nc.sync.dma_start(out=wfids_sb[:, :, :], in_=wfids[:, :, :])
            idn_sb = cp.tile([128, OUT], F32R, name="idn", tag="idn")
            nc.sync.dma_start(out=idn_sb[:, :], in_=idn[:, :])
            meta_sb = cp.tile([1, 128], I32, name="meta", tag="meta")
            nc.sync.dma_start(out=meta_sb[:, :], in_=meta[:, :])

            # ---- phase 1: AjT / AjC / A0 per (batch, d-tile) --------------
            ajt = [[None] * NDT for _ in range(B)]
            ajc = [[None] * NDT for _ in range(B)]
            a0 = [[None] * NDT for _ in range(B)]

            e1v = [None, None]
            if "dyn" in parts:
                e1regs = [nc.vector.alloc_register(f"e1r{b}") for b in range(B)]
                for b in range(B):
                    nc.vector.reg_load(e1regs[b], meta_sb[0:1, 64 + b:65 + b])
                    e1v[b] = nc.s_assert_within(
                        nc.vector.snap(e1regs[b]), 0, 253,
                        skip_runtime_assert=True)

            with tc.tile_pool(name="ps1", bufs=2, space="PSUM") as ps1:
                for b in range(B):
                    for dt in range(NDT):
                        dw = DT_W[dt]
                        dwe = dw + 1 if dt == NDT - 1 else dw  # incl. ones row
                        d0 = 128 * dt
                        pa = ps1.tile([128, 256], F32, name="pa", tag="pa")
                        for k in range(HK):
                            nc.tensor.matmul(
                                pa[0:dw, 0:256],
                                w1b_sb[k][:, d0:d0 + dw],
                                tokt_sb[b][k][:, 0:256],
                                start=(k == 0), stop=(k == HK - 1))
                        t_ajt = cp.tile([128, 512], BF16,
                                        name=f"ajt{b}{dt}", tag=f"ajt{b}{dt}")
                        if dt == NDT - 1:
                            nc.vector.memset(t_ajt[0:dwe, 0:512], 0.0)
                        else:
                            nc.vector.memset(t_ajt[0:dw, 256:512], 0.0)
                        nc.scalar.activation(t_ajt[0:dw, 0:256], pa[0:dw, 0:256],
                                             AF.Copy)
                        ajt[b][dt] = t_ajt

                        t_ajc = cp.tile([128, 512], BF16,
                                        name=f"ajc{b}{dt}", tag=f"ajc{b}{dt}")
                        nc.vector.tensor_scalar(
                            t_ajc[0:dwe, 0:253], t_ajt[0:dwe, 0:253],
                            wind_sb[0:dwe, dt:dt + 1], None, AL.add)
                        nc.vector.memset(t_ajc[0:dwe, 253:512], 0.0)
                        if "dyn" in parts:
                            nc.vector.tensor_copy(
                                t_ajc[0:dwe, bass.ds(e1v[b], 254)],
                                t_ajt[0:dwe, bass.ds(e1v[b], 254)])
                        ajc[b][dt] = t_ajc

                        pi = ps1.tile([128, RPC], F32, name="pi", tag="pi")
                        for k in range(HK):
                            nc.tensor.matmul(
                                pi[0:dw, 0:RPC],
                                w1a_sb[k][:, d0:d0 + dw],
                                toki_sb[b][k][:, 0:RPC],
                                start=(k == 0), stop=(k == HK - 1))
                        t_a0 = cp.tile([128, RPC], F32,
                                       name=f"a0{b}{dt}", tag=f"a0{b}{dt}")
                        if dt == NDT - 1:
                            nc.vector.memset(t_a0[0:dwe, :], 1.0)
                        nc.vector.tensor_scalar(
                            t_a0[0:dw, :], pi[0:dw, :],
                            b1c_sb[0:dw, dt:dt + 1], None, AL.add)
                        a0[b][dt] = t_a0

            # ---- phase 3 emitted early so fid matmuls fill PE bubbles -----
            _ps2cm = tc.tile_pool(name="ps2", bufs=2, space="PSUM")
            ps2 = _ps2cm.__enter__()
            if "fid" in parts:
                psf = ps2.tile([2, FOUT], F32, name="psf", tag="psf", bufs=1)
                for k in range(FKT):
                    nc.tensor.matmul(psf[:, :], inpt_sb[:, k, :],
                                     wfids_sb[:, k, :],
                                     start=(k == 0), stop=(k == FKT - 1))
                outf_sb = pp.tile([2, FOUT], F32, name="outf", tag="outf")
                nc.vector.tensor_copy(outf_sb[:, :], psf[:, :])
                nc.sync.dma_start(out=outf[:, :], in_=outf_sb[:, :])

            # ---- phase 2: span units (2 rows x 253 spans, per batch) ------
            vregs = [nc.vector.alloc_register(f"vdyn{rr}") for rr in range(2)]
            units = range(16) if "p2" in parts else range(0)
            for u in units:
                for b in range(B):
                    hts = [wp.tile([128, 2, hblk], BF16,
                                   name=f"h{dt}", tag=f"h{dt}")
                           for dt in range(NDT)]
                    vload = [None, None]
                    if "dyn" in parts:
                        nc.vector.reg_load(
                            vregs,
                            meta_sb[0:1, b * RPC + 2 * u:b * RPC + 2 * u + 2])
                        for rr in range(2):
                            vload[rr] = nc.s_assert_within(
                                nc.vector.snap(vregs[rr]), 0, 253,
                                skip_runtime_assert=True)
                    for rr in range(2):
                        r = 2 * u + rr
                        for dt in range(NDT):
                            dwe = DT_W[dt] + 1 if dt == NDT - 1 else DT_W[dt]
                            nc.vector.tensor_scalar(
                                hts[dt][0:dwe, rr, 0:254],
                                ajt[b][dt][0:dwe, 0:254],
                                a0[b][dt][0:dwe, r:r + 1], 0.0,
                                AL.add, AL.max)
                        if "dyn" in parts:
                            for dt in range(NDT):
                                dwe = DT_W[dt] + 1 if dt == NDT - 1 else DT_W[dt]
                                nc.vector.tensor_scalar(
                                    hts[dt][0:dwe, rr, bass.ds(vload[rr], wb)],
                                    ajc[b][dt][0:dwe, bass.ds(vload[rr], wb)],
                                    a0[b][dt][0:dwe, r:r + 1], 0.0,
                                    AL.add, AL.max)

                    psl = ps2.tile([OUT, 508], F32, name="psl", tag="psl")
                    for dt in range(NDT):
                        kw = DT_W[dt] + 1 if dt == NDT - 1 else DT_W[dt]
                        nc.tensor.matmul(
                            psl[:, :],
                            w2t_sb[0:kw, dt, :],
                            hts[dt][0:kw, :, 0:254],
                            start=(dt == 0), stop=(dt == NDT - 1))

                    sl = pp.tile([OUT, 512], F32R, name="sl", tag="sl")
                    nc.scalar.activation(sl[0:OUT, 0:508], psl[:, :], AF.Copy)

                    pst = ps2.tile([128, 4 * OUT], F32R, name="pst", tag="pst")
                    for ck in range(4):
                        cw = 128 if ck < 3 else 508 - 3 * 128
                        nc.tensor.transpose(
                            pst[0:cw, OUT * ck:OUT * (ck + 1)],
                            sl[0:OUT, 128 * ck:128 * ck + cw],
                            idn_sb[0:OUT, 0:OUT])

                    if "post" not in parts:
                        continue
                    se = pp.tile([128, 4], F32, name="se", tag="se")
                    nc.vector.memset(se[:, :], 1.0)
                    et = pp.tile([128, 4 * OUT], F32, name="et", tag="et")
                    for ck in range(4):
                        cw = 128 if ck < 3 else 508 - 3 * 128
                        nc.scalar.activation(
                            et[0:cw, OUT * ck:OUT * (ck + 1)],
                            pst[0:cw, OUT * ck:OUT * (ck + 1)],
                            AF.Exp, accum_out=se[0:cw, ck:ck + 1])
                    lns = pp.tile([128, 4], F32, name="lns", tag="lns")
                    nc.scalar.activation(lns[:, :], se[:, :], AF.Ln)

                    av = pp.tile([128, 4], F32, name="av", tag="av")
                    nc.sync.dma_start(out=av[:, :], in_=avp[u])
                    t1 = pp.tile([128, 4], F32, name="t1", tag="t1")
                    nc.vector.tensor_tensor(t1[:, :], av[:, :], lns[:, :],
                                            AL.mult)
                    s1 = pp.tile([128, 4], F32, name="s1", tag="s1")
                    nc.vector.tensor_tensor(s1[:, :], av[:, :], t1[:, :],
                                            AL.subtract)
                    nc.vector.tensor_scalar(s1[:, :], s1[:, :], -1.0, None,
                                            AL.add)

                    ot = pp.tile([128, 4 * OUT], F32, name="ot", tag="ot")
                    for ck in range(4):
                        cw = 128 if ck < 3 else 508 - 3 * 128
                        nc.vector.tensor_scalar(
                            ot[0:cw, OUT * ck:OUT * (ck + 1)],
                            pst[0:cw, OUT * ck:OUT * (ck + 1)],
                            av[0:cw, ck:ck + 1], s1[0:cw, ck:ck + 1],
                            AL.mult, AL.add)
                    base = 508 * u
                    nc.sync.dma_start(
                        out=outr[base:base + 384, b, :].rearrange(
                            "(c p) o -> p c o", p=128),
                        in_=ot[0:128, 0:120].rearrange("p (c o) -> p c o", o=OUT))
                    nc.sync.dma_start(
                        out=outr[base + 384:base + 508, b, :],
                        in_=ot[0:124, 120:160])

            _ps2cm.__exit__(None, None, None)

    nc.compile()
    return nc


def _host_prep(hidden, pred_spans, token_nums, span_avail, fid_vecs,
               W1, b1, W2, b2, Wfid, bfid):
    f32 = np.float32
    bf16 = ml_dtypes.bfloat16
    hidden = np.ascontiguousarray(hidden, dtype=f32)
    span_avail = np.asarray(span_avail)
    fid_vecs = np.asarray(fid_vecs, dtype=f32)
    W1 = np.asarray(W1, dtype=f32)
    b1 = np.asarray(b1, dtype=f32)
    W2 = np.asarray(W2, dtype=f32)
    b2 = np.asarray(b2, dtype=f32)
    Wfid = np.asarray(Wfid, dtype=f32)
    bfid = np.asarray(bfid, dtype=f32)
    s0, e0 = int(pred_spans[0, 0]), int(pred_spans[0, 1])
    s1_, e1_ = int(pred_spans[1, 0]), int(pred_spans[1, 1])
    spans = [(s0, e0), (s1_, e1_)]

    # correction width bucket
    wneed = 1
    for (s, e) in spans:
        jhi = min(e, M - 1)
        if 0 <= s <= jhi:
            wneed = max(wneed, jhi - s + 1)
    wb = 8
    while wb < wneed:
        wb *= 2
    wb = min(wb, 256)

    tok = hidden[:, 1:M + 1, :]                       # [2, 253, 768]
    tokt_full = np.zeros((B, NH, 256), f32)
    tokt_full[:, :, :M] = np.transpose(tok, (0, 2, 1))
    TOKT = np.ascontiguousarray(
        tokt_full.reshape(B, HK, 128, 256).astype(bf16))

    W1A = np.ascontiguousarray(W1[0:NH].reshape(HK, 128, D1).astype(bf16))
    W1B = np.ascontiguousarray(W1[NH:2 * NH].reshape(HK, 128, D1).astype(bf16))
    w_ind = W1[2 * NH]                                # [770]

    W2T = np.zeros((128, NDT, OUT), f32)
    for dt in range(6):
        W2T[:, dt, :] = W2[128 * dt:128 * (dt + 1), :]
    W2T[0, 6, :] = W2[768, :]
    W2T[1, 6, :] = W2[769, :]
    W2T[2, 6, :] = b2
    W2T = W2T.astype(bf16)

    WIND = np.zeros((128, NDT), f32)
    B1C = np.zeros((128, NDT), f32)
    for dt in range(NDT):
        dw = DT_W[dt]
        WIND[:dw, dt] = w_ind[128 * dt:128 * dt + dw]
        B1C[:dw, dt] = b1[128 * dt:128 * dt + dw]

    IDN = np.zeros((128, OUT), f32)
    IDN[np.arange(OUT), np.arange(OUT)] = 1.0

    # fid head input (host): cls, masked mean, fid_vecs, ones
    Lpad = hidden.shape[1]
    p = np.arange(Lpad)
    t = np.asarray(token_nums).astype(np.int64)
    dis = np.array([e0 - s0, e1_ - s1_], np.int64)
    mask = ((p[None, :] >= t[:, None] + 2)
            & (p[None, :] <= t[:, None] + 2 + dis[:, None])).astype(np.float64)
    denom = mask.sum(axis=1, keepdims=True)
    pred_vecs = (np.einsum('bl,blh->bh', mask, hidden.astype(np.float64))
                 / denom)
    cls = hidden[:, 0, :].astype(np.float64)
    inp_aug = np.concatenate(
        [cls, pred_vecs, fid_vecs.astype(np.float64),
         np.ones((B, 1))], axis=1).astype(f32)       # [2, 2237]
    INPT = np.zeros((128, FKT, 2), f32)
    for k in range(FKT):
        rows = inp_aug[:, 128 * k:128 * (k + 1)]      # [2, <=128]
        INPT[:rows.shape[1], k, :] = rows.T
    INPT = INPT.astype(bf16)

    Wfid_aug = np.concatenate([Wfid, bfid[None, :]], axis=0)  # [2237, 700]

    # per-core tensors
    av_flat = (span_avail.reshape(-1) == 1)

    per_core = []
    for c in range(NCORE):
        gi0 = RPC * c
        TOKI = np.zeros((B, HK, 128, RPC), f32)
        for r in range(RPC):
            gi = gi0 + r
            if gi < M:
                TOKI[:, :, :, r] = hidden[:, gi + 1, :].reshape(B, HK, 128)
        TOKI = TOKI.astype(bf16)

        META = np.zeros((1, 128), np.int32)
        META[0, :] = SAFE
        for b, (s, e) in enumerate(spans):
            jhi = min(e, M - 1)
            for r in range(RPC):
                gi = gi0 + r
                v = gi if (s <= gi <= jhi) else SAFE
                META[0, b * RPC + r] = v
            META[0, 64 + b] = max(0, min(e + 1, M))

        AVP = np.zeros((16, 128, 4), f32)
        for u in range(16):
            for ck in range(4):
                cw = 128 if ck < 3 else 508 - 384
                for pi in range(cw):
                    ls = 128 * ck + pi          # position within the 508-col unit
                    rr, j = ls // 254, ls % 254
                    gi = gi0 + 2 * u + rr
                    if j < M and gi < M and av_flat[gi * M + j]:
                        AVP[u, pi, ck] = 1.0

        WFIDS = np.zeros((128, FKT, FOUT), f32)
        cols = Wfid_aug[:, FOUT * c:min(FOUT * (c + 1), FID)]  # [2237, <=88]
        for k in range(FKT):
            rows = cols[128 * k:128 * (k + 1)]
            WFIDS[:rows.shape[0], k, :rows.shape[1]] = rows
        WFIDS = WFIDS.astype(bf16)

        per_core.append(dict(
            TOKT=TOKT, TOKI=TOKI, W1A=W1A, W1B=W1B, W2T=W2T, WIND=WIND,
            B1C=B1C, AVP=AVP, INPT=INPT, WFIDS=WFIDS, IDN=IDN, META=META))

    aux = dict(spans=spans, tok=tok, w_ind=w_ind, W1=W1, b1=b1, W2=W2, b2=b2,
               av_flat=av_flat)
    return wb, per_core, aux


def _exact_cell_fix(results, aux):
    """The device computes the exact span cell (s_b, e_b) with ind=1 (inside).
    Recompute it exactly on host with ind=2 and overwrite."""
    tok = aux["tok"].astype(np.float64)
    W1 = aux["W1"].astype(np.float64)
    for b, (s, e) in enumerate(aux["spans"]):
        if not (0 <= s < M and 0 <= e < M):
            continue
        n = s * M + e
        if not aux["av_flat"][n]:
            continue
        ai = tok[b, s] @ W1[0:NH]
        aj = tok[b, e] @ W1[NH:2 * NH]
        h = ai + aj + 2.0 * aux["w_ind"].astype(np.float64) \
            + aux["b1"].astype(np.float64)
        h = np.maximum(h, 0.0)
        logits = h @ aux["W2"].astype(np.float64) + aux["b2"].astype(np.float64)
        mx = logits.max()
        logp = logits - (mx + np.log(np.exp(logits - mx).sum()))
        results[n, b, :] = logp.astype(np.float32)


def kernel(hidden, pred_spans, token_nums, span_avail, fid_vecs,
           W1, b1, W2, b2, Wfid, bfid):
    wb, per_core, aux = _host_prep(
        hidden, pred_spans, token_nums, span_avail, fid_vecs,
        W1, b1, W2, b2, Wfid, bfid)

    if wb not in _CACHE:
        _CACHE[wb] = _build(wb)
    nc = _CACHE[wb]

    res = run_bass_kernel_spmd(nc, per_core, list(range(NCORE)))

    parts = []
    for c in range(NCORE):
        o = res.results[c]["OUTR"].reshape(RPC, 254, B, OUT)[:, :M]
        parts.append(o.reshape(RPC * M, B, OUT))
    results = np.ascontiguousarray(
        np.concatenate(parts, axis=0)[:M * M])
    _exact_cell_fix(results, aux)

    fl = np.concatenate(
        [res.results[c]["OUTF"] for c in range(NCORE)], axis=1)[:, :FID]
    fl64 = fl.astype(np.float64)
    mx = fl64.max(axis=1, keepdims=True)
    lse = mx + np.log(np.exp(fl64 - mx).sum(axis=1, keepdims=True))
    results_fid = (fl64 - lse).astype(np.float32)

    return results, results_fid


# revision 17
# speedup vs baseline: 1.2759x; 1.1422x over previous
"""Trainium2 Bass kernel for nn_BertClassifier (span classifier + frame-id head).

Contract: kernel(**inputs) takes the FULL unsharded inputs (as produced by the
reference setup) and returns the full outputs (results [64009, 2, 40],
results_fid [2, 700]), matching reference.reference(**inputs).

Sharding: span-start rows i (253, padded to 256) are split 32-per-core across
8 NeuronCores. Each core computes, for its rows and both batch elements,
  h[d, (i,j)] = relu(AjT[d, j] + AiT[d, i] + ind(i,j)*w_ind[d] + b1[d])
  logitsT = W2T_k-tiles @ h  (PSUM accumulate over 7 k-tiles of D1=770)
followed by an on-chip transpose, log-softmax along the 40-way label axis and
the span_avail masking. The tiny frame-id matmul is sharded over its 700
output columns (88 per core); its log-softmax runs on host.

The h pipeline runs in bf16 (PE streams 1 col/cycle, DVE hits its 4x mode);
accumulation stays fp32 in PSUM. The span indicator corrections use
host-prepared dynamic-slice offsets (META) consumed via reused DVE registers.
"""

import numpy as np
import ml_dtypes

import concourse.bass as bass
import concourse.mybir as mybir
from concourse import bacc
from concourse.tile import TileContext
from concourse.bass_utils import run_bass_kernel_spmd

F32 = mybir.dt.float32
F32R = mybir.dt.float32r
BF16 = mybir.dt.bfloat16
I32 = mybir.dt.int32
AL = mybir.AluOpType
AF = mybir.ActivationFunctionType

M = 253          # real span rows/cols
B = 2
NH = 768         # BERT hidden
D1 = 770         # span MLP hidden
OUT = 40
FID = 700
RPC = 32         # rows per core
NCORE = 8
HK = 6           # 768 / 128 k-tiles for the A-matmuls
NDT = 7          # d-tiles of D1 (6x128 + 2)
DT_W = [128, 128, 128, 128, 128, 128, 2]
FKT = 18         # ceil(2237/128)
FOUT = 88        # fid output columns per core (8*88 = 704 >= 700)
SAFE = 253       # dyn-slice start that lands in pad space

_CACHE: dict = {}


def _build(wb: int, parts=("p1", "fid", "p2", "dyn", "post")):
    """Build + compile the (uniform, SPMD) Bacc program for correction-width
    bucket `wb`."""
    hblk = 256 + wb  # free-dim width of one row block inside an H tile

    nc = bacc.Bacc()

    tokt = nc.declare_dram_parameter("TOKT", [B, HK, 128, 256], BF16, isOutput=False)
    toki = nc.declare_dram_parameter("TOKI", [B, HK, 128, RPC], BF16, isOutput=False)
    w1a = nc.declare_dram_parameter("W1A", [HK, 128, D1], BF16, isOutput=False)
    w1b = nc.declare_dram_parameter("W1B", [HK, 128, D1], BF16, isOutput=False)
    w2t = nc.declare_dram_parameter("W2T", [128, NDT, OUT], BF16, isOutput=False)
    wind = nc.declare_dram_parameter("WIND", [128, NDT], F32, isOutput=False)
    b1c = nc.declare_dram_parameter("B1C", [128, NDT], F32, isOutput=False)
    avp = nc.declare_dram_parameter("AVP", [16, 128, 4], F32, isOutput=False)
    inpt = nc.declare_dram_parameter("INPT", [128, FKT, 2], BF16, isOutput=False)
    wfids = nc.declare_dram_parameter("WFIDS", [128, FKT, FOUT], BF16, isOutput=False)
    idn = nc.declare_dram_parameter("IDN", [128, OUT], F32R, isOutput=False)
    meta = nc.declare_dram_parameter("META", [1, 128], I32, isOutput=False)

    outr = nc.declare_dram_parameter("OUTR", [RPC * 254, B, OUT], F32, isOutput=True)
    outf = nc.declare_dram_parameter("OUTF", [2, FOUT], F32, isOutput=True)

    with TileContext(nc) as tc:
        with tc.tile_pool(name="const", bufs=1) as cp, \
             tc.tile_pool(name="work", bufs=2) as wp, \
             tc.tile_pool(name="post", bufs=2) as pp:

            # ---- constant loads -------------------------------------------
            tokt_sb = [[cp.tile([128, 256], BF16, name=f"tokt{b}{k}", tag=f"tokt{b}{k}")
                        for k in range(HK)] for b in range(B)]
            toki_sb = [[cp.tile([128, RPC], BF16, name=f"toki{b}{k}", tag=f"toki{b}{k}")
                        for k in range(HK)] for b in range(B)]
            w1a_sb = [cp.tile([128, D1], BF16, name=f"w1a{k}", tag=f"w1a{k}")
                      for k in range(HK)]
            w1b_sb = [cp.tile([128, D1], BF16, name=f"w1b{k}", tag=f"w1b{k}")
                      for k in range(HK)]
            for b in range(B):
                for k in range(HK):
                    nc.sync.dma_start(out=tokt_sb[b][k][:, :], in_=tokt[b, k])
                    nc.sync.dma_start(out=toki_sb[b][k][:, :], in_=toki[b, k])
            for k in range(HK):
                nc.sync.dma_start(out=w1a_sb[k][:, :], in_=w1a[k])
                nc.sync.dma_start(out=w1b_sb[k][:, :], in_=w1b[k])
            w2t_sb = cp.tile([128, NDT, OUT], BF16, name="w2t", tag="w2t")
            nc.sync.dma_start(out=w2t_sb[:, :, :], in_=w2t[:, :, :])
            wind_sb = cp.tile([128, NDT], F32, name="wind", tag="wind")
            nc.sync.dma_start(out=wind_sb[:, :], in_=wind[:, :])
            b1c_sb = cp.tile([128, NDT], F32, name="b1c", tag="b1c")
            nc.sync.dma_start(out=b1c_sb[:, :], in_=b1c[:, :])
            inpt_sb = cp.tile([128, FKT, 2], BF16, name="inpt", tag="inpt")
            nc.sync.dma_start(out=inpt_sb[:, :, :], in_=inpt[:, :, :])
            wfids_sb = cp.tile([128, FKT, FOUT], BF16, name="wfids", tag="wfids")
            nc.sync.dma_start(out=wfids_sb[:, :, :], in_=wfids[:, :, :])
            idn_sb = cp.tile([128, OUT], F32R, name="idn", tag="idn")
            nc.sync.dma_start(out=idn_sb[:, :], in_=idn[:, :])
            meta_sb = cp.tile([1, 128], I32, name="meta", tag="meta")
            nc.sync.dma_start(out=meta_sb[:, :], in_=meta[:, :])

            # ---- phase 1: AjT / AjC / A0 combined tiles per batch ---------
            ajt_all = [None, None]   # [128, 7, 512] bf16
            ajc_all = [None, None]   # [128, 7, 512] bf16
            a0_all = [None, None]    # [128, 7, 32] f32

            e1v = [None, None]
            if "dyn" in parts:
                e1regs = [nc.vector.alloc_register(f"e1r{b}") for b in range(B)]
                for b in range(B):
                    nc.vector.reg_load(e1regs[b], meta_sb[0:1, 64 + b:65 + b])
                    e1v[b] = nc.s_assert_within(
                        nc.vector.snap(e1regs[b]), 0, 253,
                        skip_runtime_assert=True)

            with tc.tile_pool(name="ps1", bufs=2, space="PSUM") as ps1:
                for b in range(B):
                    t_ajt = cp.tile([128, NDT, 512], BF16,
                                    name=f"ajta{b}", tag=f"ajta{b}")
                    t_ajc = cp.tile([128, NDT, 512], BF16,
                                    name=f"ajca{b}", tag=f"ajca{b}")
                    t_a0 = cp.tile([128, NDT, RPC], F32,
                                   name=f"a0a{b}", tag=f"a0a{b}")
                    ajt_all[b], ajc_all[b], a0_all[b] = t_ajt, t_ajc, t_a0
                    for dt in range(NDT):
                        dw = DT_W[dt]
                        d0 = 128 * dt
                        pa = ps1.tile([128, 256], F32, name="pa", tag="pa")
                        for k in range(HK):
                            nc.tensor.matmul(
                                pa[0:dw, 0:256],
                                w1b_sb[k][:, d0:d0 + dw],
                                tokt_sb[b][k][:, 0:256],
                                start=(k == 0), stop=(k == HK - 1))
                        if dt == NDT - 1:
                            nc.vector.memset(t_ajt[:, dt, 0:512], 0.0)
                        else:
                            nc.vector.memset(t_ajt[:, dt, 256:512], 0.0)
                        nc.scalar.activation(t_ajt[0:dw, dt, 0:256],
                                             pa[0:dw, 0:256], AF.Copy)

                        # AjC = AjT + w_ind on [0, e+1), AjT elsewhere
                        nc.vector.tensor_scalar(
                            t_ajc[:, dt, 0:253], t_ajt[:, dt, 0:253],
                            wind_sb[:, dt:dt + 1], None, AL.add)
                        nc.vector.memset(t_ajc[:, dt, 253:512], 0.0)
                        if "dyn" in parts:
                            nc.vector.tensor_copy(
                                t_ajc[:, dt, bass.ds(e1v[b], 254)],
                                t_ajt[:, dt, bass.ds(e1v[b], 254)])

                        pi = ps1.tile([128, RPC], F32, name="pi", tag="pi")
                        for k in range(HK):
                            nc.tensor.matmul(
                                pi[0:dw, 0:RPC],
                                w1a_sb[k][:, d0:d0 + dw],
                                toki_sb[b][k][:, 0:RPC],
                                start=(k == 0), stop=(k == HK - 1))
                        if dt == NDT - 1:
                            nc.vector.memset(t_a0[:, dt, :], 1.0)
                            nc.vector.tensor_scalar(
                                t_a0[0:2, dt, :], pi[0:2, :],
                                b1c_sb[0:2, dt:dt + 1], None, AL.add)
                        else:
                            nc.vector.tensor_scalar(
                                t_a0[:, dt, :], pi[:, :],
                                b1c_sb[:, dt:dt + 1], None, AL.add)

            # ---- phase 3 emitted early so fid matmuls fill PE bubbles -----
            _ps2cm = tc.tile_pool(name="ps2", bufs=2, space="PSUM")
            ps2 = _ps2cm.__enter__()
            if "fid" in parts:
                psf = ps2.tile([2, FOUT], F32, name="psf", tag="psf", bufs=1)
                for k in range(FKT):
                    nc.tensor.matmul(psf[:, :], inpt_sb[:, k, :],
                                     wfids_sb[:, k, :],
                                     start=(k == 0), stop=(k == FKT - 1))
                outf_sb = pp.tile([2, FOUT], F32, name="outf", tag="outf")
                nc.vector.tensor_copy(outf_sb[:, :], psf[:, :])
                nc.sync.dma_start(out=outf[:, :], in_=outf_sb[:, :])

            # ---- phase 2: span units (2 rows x 253 spans, per batch) ------
            vregs = [nc.vector.alloc_register(f"vdyn{rr}") for rr in range(2)]
            units = range(16) if "p2" in parts else range(0)
            for u in units:
                av = pp.tile([128, 4], F32, name="av", tag="av")
                nc.sync.dma_start(out=av[:, :], in_=avp[u])
                seg = pp.tile([128, 8], F32, name="seg", tag="seg")
                pstl = [None, None]
                for b in range(B):
                    h_all = wp.tile([128, NDT, 2, hblk], BF16,
                                    name="h", tag="h")
                    vload = [None, None]
                    if "dyn" in parts:
                        nc.vector.reg_load(
                            vregs,
                            meta_sb[0:1, b * RPC + 2 * u:b * RPC + 2 * u + 2])
                        for rr in range(2):
                            vload[rr] = nc.s_assert_within(
                                nc.vector.snap(vregs[rr]), 0, 253,
                                skip_runtime_assert=True)
                    for rr in range(2):
                        r = 2 * u + rr
                        for dt in range(NDT):
                            nc.vector.tensor_scalar(
                                h_all[:, dt, rr, 0:256],
                                ajt_all[b][:, dt, 0:256],
                                a0_all[b][:, dt, r:r + 1], 0.0,
                                AL.add, AL.max)
                        if "dyn" in parts:
                            a0b = a0_all[b][:, :, r:r + 1].broadcast_to(
                                [128, NDT, wb])
                            nc.vector.tensor_tensor(
                                h_all[:, :, rr, bass.ds(vload[rr], wb)],
                                ajc_all[b][:, :, bass.ds(vload[rr], wb)],
                                a0b, AL.add)
                            nc.vector.tensor_scalar(
                                h_all[:, :, rr, bass.ds(vload[rr], wb)],
                                h_all[:, :, rr, bass.ds(vload[rr], wb)],
                                0.0, None, AL.max)

                    psl = ps2.tile([OUT, 508], F32, name="psl", tag="psl")
                    for dt in range(NDT):
                        kw = DT_W[dt] + 1 if dt == NDT - 1 else DT_W[dt]
                        nc.tensor.matmul(
                            psl[:, :],
                            w2t_sb[0:kw, dt, :],
                            h_all[0:kw, dt, :, 0:254],
                            start=(dt == 0), stop=(dt == NDT - 1))

                    sl = pp.tile([OUT, 512], F32R, name="sl", tag="sl")
                    nc.scalar.activation(sl[0:OUT, 0:508], psl[:, :], AF.Copy)
                    nc.vector.memset(sl[0:OUT, 508:512].bitcast(F32), 0.0)

                    pst = ps2.tile([128, 4 * OUT], F32R, name="pst", tag="pst",
                                   bufs=3)
                    for ck in range(4):
                        nc.tensor.transpose(
                            pst[:, OUT * ck:OUT * (ck + 1)],
                            sl[0:OUT, 128 * ck:128 * (ck + 1)],
                            idn_sb[0:OUT, 0:OUT])
                    pstl[b] = pst

                    if "post" not in parts:
                        continue
                    et = pp.tile([128, 4 * OUT], F32, name="et", tag="et")
                    nc.scalar.activation(et[:, :], pst[:, :], AF.Exp)
                    nc.vector.tensor_reduce(
                        seg[:, 4 * b:4 * b + 4],
                        et[:, :].rearrange("p (c o) -> p c o", o=OUT),
                        mybir.AxisListType.X, AL.add)

                if "post" not in parts:
                    continue
                lns = pp.tile([128, 8], F32, name="lns", tag="lns")
                nc.scalar.activation(lns[:, :], seg[:, :], AF.Ln)

                for b in range(B):
                    t1 = pp.tile([128, 4], F32, name="t1", tag="t1")
                    nc.vector.tensor_tensor(t1[:, :], av[:, :],
                                            lns[:, 4 * b:4 * b + 4], AL.mult)
                    s1 = pp.tile([128, 4], F32, name="s1", tag="s1")
                    nc.vector.tensor_tensor(s1[:, :], av[:, :], t1[:, :],
                                            AL.subtract)
                    nc.vector.tensor_scalar(s1[:, :], s1[:, :], -1.0, None,
                                            AL.add)

                    ot = pp.tile([128, 4 * OUT], F32, name="ot", tag="ot")
                    otv = ot[:, :].rearrange("p (c o) -> p c o", o=OUT)
                    pstv = pstl[b][:, :].rearrange("p (c o) -> p c o", o=OUT)
                    avb = av[:, :].rearrange("p (c x) -> p c x", x=1)
                    avb = avb.broadcast_to([128, 4, OUT])
                    s1b = s1[:, :].rearrange("p (c x) -> p c x", x=1)
                    s1b = s1b.broadcast_to([128, 4, OUT])
                    nc.vector.tensor_tensor(otv, pstv, avb, AL.mult)
                    nc.vector.tensor_tensor(otv, otv, s1b, AL.add)
                    base = 508 * u
                    nc.sync.dma_start(
                        out=outr[base:base + 384, b, :].rearrange(
                            "(c p) o -> p c o", p=128),
                        in_=ot[0:128, 0:120].rearrange("p (c o) -> p c o", o=OUT))
                    nc.sync.dma_start(
                        out=outr[base + 384:base + 508, b, :],
                        in_=ot[0:124, 120:160])

            _ps2cm.__exit__(None, None, None)

    nc.compile()
    return nc


def _host_prep(hidden, pred_spans, token_nums, span_avail, fid_vecs,
               W1, b1, W2, b2, Wfid, bfid):
    f32 = np.float32
    bf16 = ml_dtypes.bfloat16
    hidden = np.ascontiguousarray(hidden, dtype=f32)
    span_avail = np.asarray(span_avail)
    fid_vecs = np.asarray(fid_vecs, dtype=f32)
    W1 = np.asarray(W1, dtype=f32)
    b1 = np.asarray(b1, dtype=f32)
    W2 = np.asarray(W2, dtype=f32)
    b2 = np.asarray(b2, dtype=f32)
    Wfid = np.asarray(Wfid, dtype=f32)
    bfid = np.asarray(bfid, dtype=f32)
    s0, e0 = int(pred_spans[0, 0]), int(pred_spans[0, 1])
    s1_, e1_ = int(pred_spans[1, 0]), int(pred_spans[1, 1])
    spans = [(s0, e0), (s1_, e1_)]

    # correction width bucket
    wneed = 1
    for (s, e) in spans:
        jhi = min(e, M - 1)
        if 0 <= s <= jhi:
            wneed = max(wneed, jhi - s + 1)
    wb = 8
    while wb < wneed:
        wb *= 2
    wb = min(wb, 256)

    tok = hidden[:, 1:M + 1, :]                       # [2, 253, 768]
    tokt_full = np.zeros((B, NH, 256), f32)
    tokt_full[:, :, :M] = np.transpose(tok, (0, 2, 1))
    TOKT = np.ascontiguousarray(
        tokt_full.reshape(B, HK, 128, 256).astype(bf16))

    W1A = np.ascontiguousarray(W1[0:NH].reshape(HK, 128, D1).astype(bf16))
    W1B = np.ascontiguousarray(W1[NH:2 * NH].reshape(HK, 128, D1).astype(bf16))
    w_ind = W1[2 * NH]                                # [770]

    W2T = np.zeros((128, NDT, OUT), f32)
    for dt in range(6):
        W2T[:, dt, :] = W2[128 * dt:128 * (dt + 1), :]
    W2T[0, 6, :] = W2[768, :]
    W2T[1, 6, :] = W2[769, :]
    W2T[2, 6, :] = b2
    W2T = W2T.astype(bf16)

    WIND = np.zeros((128, NDT), f32)
    B1C = np.zeros((128, NDT), f32)
    for dt in range(NDT):
        dw = DT_W[dt]
        WIND[:dw, dt] = w_ind[128 * dt:128 * dt + dw]
        B1C[:dw, dt] = b1[128 * dt:128 * dt + dw]

    IDN = np.zeros((128, OUT), f32)
    IDN[np.arange(OUT), np.arange(OUT)] = 1.0

    # fid head input (host): cls, masked mean, fid_vecs, ones
    Lpad = hidden.shape[1]
    p = np.arange(Lpad)
    t = np.asarray(token_nums).astype(np.int64)
    dis = np.array([e0 - s0, e1_ - s1_], np.int64)
    mask = ((p[None, :] >= t[:, None] + 2)
            & (p[None, :] <= t[:, None] + 2 + dis[:, None])).astype(np.float64)
    denom = mask.sum(axis=1, keepdims=True)
    pred_vecs = (np.einsum('bl,blh->bh', mask, hidden.astype(np.float64))
                 / denom)
    cls = hidden[:, 0, :].astype(np.float64)
    inp_aug = np.concatenate(
        [cls, pred_vecs, fid_vecs.astype(np.float64),
         np.ones((B, 1))], axis=1).astype(f32)       # [2, 2237]
    INPT = np.zeros((128, FKT, 2), f32)
    for k in range(FKT):
        rows = inp_aug[:, 128 * k:128 * (k + 1)]      # [2, <=128]
        INPT[:rows.shape[1], k, :] = rows.T
    INPT = INPT.astype(bf16)

    Wfid_aug = np.concatenate([Wfid, bfid[None, :]], axis=0)  # [2237, 700]

    # per-core tensors
    av_flat = (span_avail.reshape(-1) == 1)

    per_core = []
    for c in range(NCORE):
        gi0 = RPC * c
        TOKI = np.zeros((B, HK, 128, RPC), f32)
        for r in range(RPC):
            gi = gi0 + r
            if gi < M:
                TOKI[:, :, :, r] = hidden[:, gi + 1, :].reshape(B, HK, 128)
        TOKI = TOKI.astype(bf16)

        META = np.zeros((1, 128), np.int32)
        META[0, :] = SAFE
        for b, (s, e) in enumerate(spans):
            jhi = min(e, M - 1)
            for r in range(RPC):
                gi = gi0 + r
                v = gi if (s <= gi <= jhi) else SAFE
                META[0, b * RPC + r] = v
            META[0, 64 + b] = max(0, min(e + 1, M))

        AVP = np.zeros((16, 128, 4), f32)
        for u in range(16):
            for ck in range(4):
                cw = 128 if ck < 3 else 508 - 384
                for pi in range(cw):
                    ls = 128 * ck + pi          # position within the 508-col unit
                    rr, j = ls // 254, ls % 254
                    gi = gi0 + 2 * u + rr
                    if j < M and gi < M and av_flat[gi * M + j]:
                        AVP[u, pi, ck] = 1.0

        WFIDS = np.zeros((128, FKT, FOUT), f32)
        cols = Wfid_aug[:, FOUT * c:min(FOUT * (c + 1), FID)]  # [2237, <=88]
        for k in range(FKT):
            rows = cols[128 * k:128 * (k + 1)]
            WFIDS[:rows.shape[0], k, :rows.shape[1]] = rows
        WFIDS = WFIDS.astype(bf16)

        per_core.append(dict(
            TOKT=TOKT, TOKI=TOKI, W1A=W1A, W1B=W1B, W2T=W2T, WIND=WIND,
            B1C=B1C, AVP=AVP, INPT=INPT, WFIDS=WFIDS, IDN=IDN, META=META))

    aux = dict(spans=spans, tok=tok, w_ind=w_ind, W1=W1, b1=b1, W2=W2, b2=b2,
               av_flat=av_flat)
    return wb, per_core, aux


def _exact_cell_fix(results, aux):
    """The device computes the exact span cell (s_b, e_b) with ind=1 (inside).
    Recompute it exactly on host with ind=2 and overwrite."""
    tok = aux["tok"].astype(np.float64)
    W1 = aux["W1"].astype(np.float64)
    for b, (s, e) in enumerate(aux["spans"]):
        if not (0 <= s < M and 0 <= e < M):
            continue
        n = s * M + e
        if not aux["av_flat"][n]:
            continue
        ai = tok[b, s] @ W1[0:NH]
        aj = tok[b, e] @ W1[NH:2 * NH]
        h = ai + aj + 2.0 * aux["w_ind"].astype(np.float64) \
            + aux["b1"].astype(np.float64)
        h = np.maximum(h, 0.0)
        logits = h @ aux["W2"].astype(np.float64) + aux["b2"].astype(np.float64)
        mx = logits.max()
        logp = logits - (mx + np.log(np.exp(logits - mx).sum()))
        results[n, b, :] = logp.astype(np.float32)


def kernel(hidden, pred_spans, token_nums, span_avail, fid_vecs,
           W1, b1, W2, b2, Wfid, bfid):
    wb, per_core, aux = _host_prep(
        hidden, pred_spans, token_nums, span_avail, fid_vecs,
        W1, b1, W2, b2, Wfid, bfid)

    if wb not in _CACHE:
        _CACHE[wb] = _build(wb)
    nc = _CACHE[wb]

    res = run_bass_kernel_spmd(nc, per_core, list(range(NCORE)))

    parts = []
    for c in range(NCORE):
        o = res.results[c]["OUTR"].reshape(RPC, 254, B, OUT)[:, :M]
        parts.append(o.reshape(RPC * M, B, OUT))
    results = np.ascontiguousarray(
        np.concatenate(parts, axis=0)[:M * M])
    _exact_cell_fix(results, aux)

    fl = np.concatenate(
        [res.results[c]["OUTF"] for c in range(NCORE)], axis=1)[:, :FID]
    fl64 = fl.astype(np.float64)
    mx = fl64.max(axis=1, keepdims=True)
    lse = mx + np.log(np.exp(fl64 - mx).sum(axis=1, keepdims=True))
    results_fid = (fl64 - lse).astype(np.float32)

    return results, results_fid
